# revision 1
# baseline (speedup 1.0000x reference)
"""EpipolarCrossViewAttention TRN2 kernel (8 NeuronCores, data-parallel).

Sharding: core c -> batch b=c//2, query-row half h=c%2 (1152 query
tokens). Each core computes k/v for its batch's full 2304 keys
(duplicated across the core pair), the epipolar bias + exact top-32
mask + softmax for its own query rows, and its rows' output
projection. Host does layout only (reshape/slice/transpose + folding
bo' = bo + Wo@bv, added on host after gather).

Structure (v2, software-pipelined): the per-g-tile bias+topk work
(geometry numerator matmul -> |.|x10 -> reciprocal -> multiply ->
36x max8 candidates -> 4-round merge -> threshold -> s1m mask) runs
on DVE/Pool/Act and is interleaved with the PE-heavy LN+projection
phase and the attention phase, so all engines stay busy. The top-32
numerics match the baseline exactly (same op sequence for gb =
10|num| * (-1/den), hi/lo split f32r numerator, DVE-exact norms).
The -BIG*min(gb-t,0) drop penalty and the +gb bias are accumulated
into the logits PSUM by identity-matmuls on PE; softmax normalization
is folded after A@V (O = (P_unnorm @ V) * (1/S) per row).
"""
import numpy as np
import concourse.bass as bass
import concourse.mybir as mybir
import concourse.tile as tile
from concourse import bacc
from concourse.bass_utils import run_bass_kernel_spmd
from concourse.masks import make_identity

F32 = mybir.dt.float32
F32R = mybir.dt.float32r
BF16 = mybir.dt.bfloat16
A = mybir.AluOpType
AF = mybir.ActivationFunctionType

B, C, H, W = 4, 1024, 48, 48
N = H * W            # 2304 keys
TQ = N // 2          # 1152 query rows per core
D = 256
NC_ = C // 128       # 8 c-tiles
NI = TQ // 128       # 9 i-tiles
NJ = N // 128        # 18 key chunks
EPS = 1e-6
LN_EPS = 1e-5
SCALE = D ** -0.5
BIG = 1.5e9
TOPCW = 64           # topk chunk width -> 36 chunks, top-8 each

_CACHE = {}
BUILD_ID = 201


def _chunks(total, step=512):
    out, x = [], 0
    while x < total:
        out.append((x, min(step, total - x)))
        x += step
    return out


def build_nc():
    nc = bacc.Bacc("TRN2", target_bir_lowering=False, debug=False)

    xq_d = nc.dram_tensor("xq", [C, TQ], F32R, kind="ExternalInput")
    xkv_d = nc.dram_tensor("xkv", [C, N], F32R, kind="ExternalInput")
    dq_d = nc.dram_tensor("dq", [3, TQ], F32, kind="ExternalInput")
    mq_d = nc.dram_tensor("mq", [3, TQ], F32, kind="ExternalInput")
    dk_d = nc.dram_tensor("dk", [3, N], F32, kind="ExternalInput")
    mk_d = nc.dram_tensor("mk", [3, N], F32, kind="ExternalInput")
    pqt_d = nc.dram_tensor("pqt", [TQ, 6], F32, kind="ExternalInput")
    pkt_d = nc.dram_tensor("pkt", [N, 6], F32, kind="ExternalInput")
    wq_d = nc.dram_tensor("wqt", [C, D], F32R, kind="ExternalInput")   # Wq.T
    wk_d = nc.dram_tensor("wkt", [C, D], F32R, kind="ExternalInput")
    wv_d = nc.dram_tensor("wvt", [C, D], F32R, kind="ExternalInput")
    wo_d = nc.dram_tensor("wot", [D, C], F32R, kind="ExternalInput")   # Wo.T
    gq_d = nc.dram_tensor("gq", [C, 1], F32, kind="ExternalInput")
    bqln_d = nc.dram_tensor("bqln", [C, 1], F32R, kind="ExternalInput")
    gk_d = nc.dram_tensor("gk", [C, 1], F32, kind="ExternalInput")
    bkln_d = nc.dram_tensor("bkln", [C, 1], F32R, kind="ExternalInput")
    bq_d = nc.dram_tensor("bq", [D, 1], F32, kind="ExternalInput")
    bk_d = nc.dram_tensor("bk", [D, 1], F32, kind="ExternalInput")
    y_d = nc.dram_tensor("y", [TQ, C], F32, kind="ExternalOutput")
    krnd_d = nc.dram_tensor("krnd_scr", [128, NJ], F32, kind="Internal")
    kmn_d = nc.dram_tensor("kmn_scr", [128, NJ], F32, kind="Internal")
    qrnd_d = nc.dram_tensor("qrnd_scr", [128, NI], F32, kind="Internal")
    nonce_d = nc.dram_tensor(f"nonce{BUILD_ID}", [1, 1], F32, kind="ExternalInput")
    dnonce_d = nc.dram_tensor(f"dnonce{BUILD_ID}", [1, 1], F32, kind="ExternalOutput")
    import os as _os
    DBG = bool(_os.environ.get("KDBG"))
    if DBG:
        dbg_gb = nc.dram_tensor("dbg_gb", [128, N], F32, kind="ExternalOutput")
        dbg_t = nc.dram_tensor("dbg_t", [128, 8], F32, kind="ExternalOutput")
        dbg_s1m = nc.dram_tensor("dbg_s1m", [128, N], F32, kind="ExternalOutput")
        dbg_P = nc.dram_tensor("dbg_P", [128, N], F32, kind="ExternalOutput")
        dbg_S = nc.dram_tensor("dbg_S", [128, 1], F32, kind="ExternalOutput")
        dbg_O = nc.dram_tensor("dbg_O", [128, D], F32, kind="ExternalOutput")

    NCC = _chunks(N)          # [(0,512),(512,512),(1024,512),(1536,512),(2048,256)]

    with tile.TileContext(nc) as tc:
      with tc.tile_pool(name="pers", bufs=1) as pers:
        nt = pers.tile([1, 1], F32, tag="nonce_t")
        nc.sync.dma_start(nt[:], nonce_d[:])
        nc.sync.dma_start(dnonce_d[:], nt[:])
        ones_col = pers.tile([128, 1], F32R, tag="ones_col")
        nc.vector.memset(ones_col[:].bitcast(F32), 1.0)
        ident_f = pers.tile([128, 128], F32, tag="ident_f")
        make_identity(nc, ident_f[:])
        ident_r = pers.tile([128, 128], F32R, tag="ident_r")
        nc.vector.tensor_copy(ident_r[:], ident_f[:])
        ident_big = pers.tile([128, 128], BF16, tag="ident_big")
        nc.vector.tensor_scalar(ident_big[:], ident_f[:], BIG, None, op0=A.mult)
        ident_b = pers.tile([128, 128], BF16, tag="ident_b")
        nc.vector.tensor_copy(ident_b[:], ident_f[:])

        woall = pers.tile([128, 2 * C], F32R, tag="woall")
        wo = [woall[:, d * C:(d + 1) * C] for d in range(2)]

        # su columns: 0,1 = -s_q(dh) ; 2,3 = -s_k(dh) ; 4,5 = u_q(dh) ; 6,7 = u_k(dh)
        su = pers.tile([128, 8], F32, tag="su")
        q_T = [pers.tile([128, TQ], F32R, tag=f"qT{d}", name=f"qT{d}") for d in range(2)]
        k_T = [pers.tile([128, N], F32R, tag=f"kT{d}", name=f"kT{d}") for d in range(2)]
        VW = D + 8            # ones col at D, zero pad to keep f32r matmul happy
        V = [pers.tile([128, VW], F32R, tag=f"V{t}", name=f"V{t}") for t in range(NJ)]
        for t in range(NJ):
            nc.vector.memset(V[t][:, D:].bitcast(F32), 0.0)
            nc.vector.memset(V[t][:, D:D + 1].bitcast(F32), 1.0)
        nkneg_b = pers.tile([128, N], F32, tag="nkneg_b")
        nqe_neg = pers.tile([128, NI], F32, tag="nqe_neg")
        suT = pers.tile([1, 8 * 128], F32R, tag="suT")
        q24 = pers.tile([24, TQ], F32R, tag="q24")
        k24 = pers.tile([24, N], F32R, tag="k24")

        with tc.tile_pool(name="wpool", bufs=1) as wpool:

            wqg = [wpool.tile([128, D], F32R, tag=f"wqg{c}", name=f"wqg{c}") for c in range(NC_)]
            wkg = [wpool.tile([128, D], F32R, tag=f"wkg{c}", name=f"wkg{c}") for c in range(NC_)]
            wvall = wpool.tile([128, NC_ * D], F32R, tag="wvall")
            wv = [wvall[:, c * D:(c + 1) * D] for c in range(NC_)]
            nc.sync.dma_start(wvall[:].rearrange("p (c d) -> p c d", d=D),
                              wv_d[:].rearrange("(c p) d -> p c d", p=128))

            # ================= phase 0: weight prep =================
            with tc.tile_pool(name="w0", bufs=2) as w0, \
                 tc.tile_pool(name="ps0a", bufs=1, space="PSUM") as ps0a, \
                 tc.tile_pool(name="ps0b", bufs=2, space="PSUM") as ps0b:
                gq_c = w0.tile([128, NC_], F32, tag="gq_c")
                gk_c = w0.tile([128, NC_], F32, tag="gk_c")
                bqln_c = w0.tile([128, NC_], F32R, tag="bqln_c")
                bkln_c = w0.tile([128, NC_], F32R, tag="bkln_c")
                nc.sync.dma_start(gq_c[:], gq_d[:].rearrange("(g p) 1 -> p g", p=128))
                nc.sync.dma_start(gk_c[:], gk_d[:].rearrange("(g p) 1 -> p g", p=128))
                nc.sync.dma_start(bqln_c[:], bqln_d[:].rearrange("(g p) 1 -> p g", p=128))
                nc.sync.dma_start(bkln_c[:], bkln_d[:].rearrange("(g p) 1 -> p g", p=128))
                bqc = w0.tile([128, 2], F32, tag="bqc")
                bkc = w0.tile([128, 2], F32, tag="bkc")
                nc.sync.dma_start(bqc[:], bq_d[:].rearrange("(g p) 1 -> p g", p=128))
                nc.sync.dma_start(bkc[:], bk_d[:].rearrange("(g p) 1 -> p g", p=128))

                psu = [ps0a.tile([128, 1], F32, tag=f"psu{dh}", name=f"psu{dh}") for dh in range(2)]
                psk = [ps0a.tile([128, 1], F32, tag=f"psk{dh}", name=f"psk{dh}") for dh in range(2)]
                wtp_cm = tc.tile_pool(name="wtp", bufs=1)
                wtp = wtp_cm.__enter__()
                wqtall = wtp.tile([128, NC_ * D], F32R, tag="wqtall")
                nc.sync.dma_start(wqtall[:].rearrange("p (c d) -> p c d", d=D),
                                  wq_d[:].rearrange("(c p) d -> p c d", p=128))
                wktall = wtp.tile([128, NC_ * D], F32R, tag="wktall")
                nc.sync.dma_start(wktall[:].rearrange("p (c d) -> p c d", d=D),
                                  wk_d[:].rearrange("(c p) d -> p c d", p=128))
                for c in range(NC_):
                    wqt_c = wqtall[:, c * D:(c + 1) * D]
                    wkt_c = wktall[:, c * D:(c + 1) * D]
                    nc.vector.tensor_scalar(wqg[c][:], wqt_c[:].bitcast(F32),
                                            gq_c[:, c:c + 1], SCALE, op0=A.mult, op1=A.mult)
                    nc.vector.tensor_scalar(wkg[c][:], wkt_c[:].bitcast(F32),
                                            gk_c[:, c:c + 1], None, op0=A.mult)
                    for dh in range(2):
                        nc.tensor.matmul(psu[dh][:],
                                         wqt_c[:, dh * 128:(dh + 1) * 128].bitcast(F32),
                                         bqln_c[:, c:c + 1].bitcast(F32), start=(c == 0),
                                         stop=(c == NC_ - 1), skip_group_check=True)
                        nc.tensor.matmul(psk[dh][:],
                                         wkt_c[:, dh * 128:(dh + 1) * 128].bitcast(F32),
                                         bkln_c[:, c:c + 1].bitcast(F32), start=(c == 0),
                                         stop=(c == NC_ - 1), skip_group_check=True)
                wtp_cm.__exit__(None, None, None)

                # ================= phase 1: geometry =================
                # Norms computed EXACTLY on DVE in token-major packed layout.
                # The q24/k24 assembly DMAs are DEFERRED (emitted after the
                # k-side projection chunks) so their long waits never block
                # projection work queued behind them on the Act DGE queue.
                with tc.tile_pool(name="geo", bufs=1) as geo:
                    def norms_side(pt_d, nch):
                        gt = geo.tile([128, nch * 6], F32, tag="gt", name=f"gt{nch}")
                        nc.sync.dma_start(
                            gt[:].rearrange("p (g c) -> p g c", c=6),
                            pt_d[:].rearrange("(g p) c -> p g c", p=128))
                        sq = geo.tile([128, nch * 6], F32, tag="sq", name=f"sq{nch}")
                        nc.vector.tensor_mul(sq[:], gt[:], gt[:])
                        n2 = geo.tile([128, nch * 2], F32, tag="n2", name=f"n2{nch}")
                        nc.vector.tensor_reduce(
                            n2[:].rearrange("p (g t) -> p g t", t=2),
                            sq[:].rearrange("p (g t c) -> p g t c", t=2, c=3),
                            axis=mybir.AxisListType.X, op=A.add)
                        sn = geo.tile([128, nch * 2], F32, tag="sn", name=f"sn{nch}")
                        nc.scalar.activation(sn[:], n2[:], AF.Sqrt)
                        scr = geo.tile([128, nch * 2], F32, tag="scr", name=f"scr{nch}")
                        nc.vector.reciprocal(scr[:], sn[:])
                        nc.vector.scalar_tensor_tensor(scr[:], n2[:], 0.5, scr[:],
                                                       op0=A.mult, op1=A.mult)
                        nc.vector.scalar_tensor_tensor(sn[:], sn[:], 0.5, scr[:],
                                                       op0=A.mult, op1=A.add)
                        dv = sn[:].rearrange("p (g t) -> p g t", t=2)[:, :, 0:1]
                        nc.vector.tensor_scalar(dv, dv, EPS, None, op0=A.max)
                        rnd = geo.tile([128, nch], F32, tag="rnd", name=f"rnd{nch}")
                        nc.vector.reciprocal(rnd[:], dv)
                        return gt, sn, rnd

                    kgt, ksn, krnd = norms_side(pkt_d, NJ)
                    rdk_row = geo.tile([1, N], F32, tag="rdk_row")
                    nk_row = geo.tile([1, N], F32, tag="nk_row")
                    kmn = geo.tile([128, NJ], F32, tag="kmn")
                    nc.vector.tensor_scalar(
                        kmn[:], ksn[:].rearrange("p (g t) -> p g t", t=2)[:, :, 1:2],
                        -1.0, None, op0=A.mult)
                    nc.sync.dma_start(krnd_d[:], krnd[:])
                    nc.sync.dma_start(kmn_d[:], kmn[:])
                    nc.sync.dma_start(
                        rdk_row[0:1, :].rearrange("a (g p) -> a g p", p=128),
                        krnd_d[:].rearrange("(a p) g -> a g p", a=1))
                    nc.sync.dma_start(
                        nk_row[0:1, :].rearrange("a (g p) -> a g p", p=128),
                        kmn_d[:].rearrange("(a p) g -> a g p", a=1))
                    nc.gpsimd.partition_broadcast(nkneg_b[:], nk_row[0:1, :],
                                                  channels=128)
                    qgt, qsn, qrnd = norms_side(pqt_d, NI)
                    rdq_row = geo.tile([1, TQ], F32, tag="rdq_row")
                    nc.sync.dma_start(qrnd_d[:], qrnd[:])
                    nc.sync.dma_start(
                        rdq_row[0:1, :].rearrange("a (g p) -> a g p", p=128),
                        qrnd_d[:].rearrange("(a p) g -> a g p", a=1))
                    nc.vector.tensor_scalar(
                        nqe_neg[:],
                        qsn[:].rearrange("p (g t) -> p g t", t=2)[:, :, 1:2],
                        -1.0, None, op0=A.mult)

                    pkin = geo.tile([35, N], F32, tag="pkin")   # dk@0:3, mk@32:35
                    nc.sync.dma_start(pkin[0:3, :], dk_d[:])
                    nc.sync.dma_start(pkin[32:35, :], mk_d[:])
                    pqin = geo.tile([35, TQ], F32, tag="pqin")  # dq@0:3, mq@32:35
                    nc.sync.dma_start(pqin[0:3, :], dq_d[:])
                    nc.sync.dma_start(pqin[32:35, :], mq_d[:])

                    scr_k = geo.tile([3, N], F32, tag="scr_k")
                    nc.gpsimd.partition_broadcast(scr_k[:], rdk_row[0:1, :], channels=3)
                    nc.vector.tensor_mul(pkin[0:3, :], pkin[0:3, :], scr_k[:])   # dkh
                    scr_q = geo.tile([3, TQ], F32, tag="scr_q")
                    nc.gpsimd.partition_broadcast(scr_q[:], rdq_row[0:1, :], channels=3)
                    nc.vector.tensor_mul(pqin[0:3, :], pqin[0:3, :], scr_q[:])   # dqh

                    khl = geo.tile([35, N], F32R, tag="khl")
                    khl2 = geo.tile([35, N], F32R, tag="khl2")
                    nc.vector.tensor_scalar(khl[:], pkin[:], 1.0, None, op0=A.mult)
                    nc.vector.tensor_sub(khl2[:], pkin[:], khl[:].bitcast(F32))
                    qhl = geo.tile([35, TQ], F32R, tag="qhl")
                    qhl2 = geo.tile([35, TQ], F32R, tag="qhl2")
                    nc.vector.tensor_scalar(qhl[:], pqin[:], 1.0, None, op0=A.mult)
                    nc.vector.tensor_sub(qhl2[:], pqin[:], qhl[:].bitcast(F32))

                    for base, srct in ((0, qhl2), (6, qhl2), (12, qhl), (18, qhl)):
                        nc.sync.dma_start(q24[base:base + 3, :], srct[0:3, :])
                        nc.sync.dma_start(q24[base + 3:base + 6, :],
                                          srct[32:35, :])
                    for base, srct in ((0, khl2), (6, khl), (12, khl2), (18, khl)):
                        nc.scalar.dma_start(k24[base:base + 3, :], srct[32:35, :])
                        nc.scalar.dma_start(k24[base + 3:base + 6, :],
                                            srct[0:3, :])


                for dh in range(2):
                    pss = ps0b.tile([128, 1], F32, tag="pss")
                    for c in range(NC_):
                        nc.tensor.matmul(pss[:],
                                         wqg[c][:, dh * 128:(dh + 1) * 128].bitcast(F32),
                                         ones_col[:].bitcast(F32), start=(c == 0),
                                         stop=(c == NC_ - 1), skip_group_check=True)
                    nc.vector.tensor_scalar(su[:, dh:dh + 1], pss[:], -1.0, None, op0=A.mult)
                    pss2 = ps0b.tile([128, 1], F32, tag="pss")
                    for c in range(NC_):
                        nc.tensor.matmul(pss2[:],
                                         wkg[c][:, dh * 128:(dh + 1) * 128].bitcast(F32),
                                         ones_col[:].bitcast(F32), start=(c == 0),
                                         stop=(c == NC_ - 1), skip_group_check=True)
                    nc.vector.tensor_scalar(su[:, 2 + dh:3 + dh], pss2[:], -1.0, None, op0=A.mult)
                    nc.vector.tensor_scalar(su[:, 4 + dh:5 + dh], psu[dh][:],
                                            bqc[:, dh:dh + 1], SCALE, op0=A.add, op1=A.mult)
                    nc.vector.tensor_scalar(su[:, 6 + dh:7 + dh], psk[dh][:],
                                            bkc[:, dh:dh + 1], None, op0=A.add)

            # su columns transposed to rows (rank-1 mean-correction lhsT)
            with tc.tile_pool(name="psu2", bufs=1, space="PSUM") as psu2:
                sur = pers.tile([128, 8], F32R, tag="sur")
                nc.vector.tensor_scalar(sur[:], su[:], 1.0, None, op0=A.mult)
                psuT = psu2.tile([1, 8 * 128], F32R, tag="psuT")
                for cc in range(8):
                    nc.tensor.transpose(psuT[0:1, cc * 128:(cc + 1) * 128],
                                        sur[:, cc:cc + 1], ident_r[:])
                nc.vector.tensor_scalar(suT[:], psuT[:].bitcast(F32), 1.0, None,
                                        op0=A.mult)

            # T/A working pools open after geometry scratch is freed.
            with tc.tile_pool(name="tg", bufs=4) as tg, \
                 tc.tile_pool(name="ts1", bufs=4) as ts1, \
                 tc.tile_pool(name="tdn", bufs=1) as tdn, \
                 tc.tile_pool(name="tgb", bufs=2) as tgb, \
                 tc.tile_pool(name="tscr", bufs=2) as tscr, \
                 tc.tile_pool(name="tm8", bufs=2) as tm8, \
                 tc.tile_pool(name="psT", bufs=2, space="PSUM") as psT:
                # ============ bias+topk for one g-tile (T-phase) ============
                tref = {}

                tref_gb = {}

                def emit_T_head(g):
                    rd = tdn.tile([128, N], F32, tag="dneg")
                    nc.gpsimd.tensor_scalar(rd[:], nkneg_b[:], nqe_neg[:, g:g + 1],
                                            -EPS, op0=A.add, op1=A.add)
                    nc.vector.reciprocal(rd[:], rd[:])   # in-place: -1/den
                    gb = tgb.tile([128, N], F32, tag="gb")
                    for c0, wd in NCC:
                        pnum = psT.tile([128, 512], F32, tag="pnum")
                        nc.tensor.matmul(pnum[:, :wd],
                                         q24[:, g * 128:(g + 1) * 128],
                                         k24[:, c0:c0 + wd], start=True, stop=True)
                        nc.scalar.activation(gb[:, c0:c0 + wd], pnum[:, :wd],
                                             AF.Abs, scale=10.0)
                    nc.vector.tensor_mul(gb[:], gb[:], rd[:])   # gb = 10|num| * (-1/den)
                    tref_gb[g] = gb

                def emit_T_tail(g):
                    gb = tref_gb.pop(g)

                    cand = tscr.tile([128, (N // TOPCW) * 8], F32, tag="cand")
                    for cch in range(N // TOPCW):
                        nc.vector.max(out=cand[:, cch * 8:(cch + 1) * 8],
                                      in_=gb[:, cch * TOPCW:(cch + 1) * TOPCW])
                    m8 = tm8.tile([128, 8], F32, tag="m8")
                    scr = tscr.tile([128, (N // TOPCW) * 8], F32, tag="scr")
                    cur = cand
                    for r in range(4):
                        nc.vector.max(out=m8[:], in_=cur[:])
                        if r < 3:
                            nxt = scr if cur is cand else cand
                            nc.vector.match_replace(out=nxt[:], in_to_replace=m8[:],
                                                    in_values=cur[:], imm_value=-3.0e38)
                            cur = nxt
                    # s1m = min(gb - t, 0)  (t = 32nd largest gb in the row).
                    # Written as f32r: it feeds the BIG-identity penalty matmul;
                    # kept entries are exactly 0 and dropped ones are huge, so
                    # f32r rounding is harmless here.
                    s1m = ts1.tile([128, N], BF16, tag="s1m")
                    nc.gpsimd.tensor_scalar(s1m[:], gb[:], m8[:, 7:8], 0.0,
                                            op0=A.subtract, op1=A.min)
                    # f32r copy of the exact-f32 bias for the pL-init matmul
                    # (selection above used the exact values; the f32r
                    # rounding only shifts kept biases by <=4e-5).
                    gbr = tg.tile([128, N], BF16, tag="gbr")
                    nc.gpsimd.tensor_scalar(gbr[:], gb[:], 1.0, None, op0=A.mult)
                    if DBG and g == 0:
                        nc.sync.dma_start(dbg_gb[:], gb[:])
                        nc.sync.dma_start(dbg_t[:], m8[:])
                        pass
                    tref[g] = (gbr, s1m)

                def emit_T(g):
                    emit_T_head(g)
                    emit_T_tail(g)

                # ================= projections =================
                def project_side(x_d, width, wg, s_col0, u_col0, out_T, with_v, chunks,
                                 pre0=None):
                    h0 = 0
                    for hw in chunks:
                        with tc.tile_pool(name="px", bufs=1) as px, \
                             tc.tile_pool(name="pxs", bufs=2) as pxs, \
                             tc.tile_pool(name="pxb", bufs=1) as pxb, \
                             tc.tile_pool(name="ps2", bufs=2, space="PSUM") as ps2, \
                             tc.tile_pool(name="ps2s", bufs=1, space="PSUM") as ps2s:
                            if h0 == 0 and pre0 is not None:
                                xtall = pre0
                            else:
                                xtall = px.tile([128, NC_ * hw], F32R, tag="xtall",
                                                name=f"xtall{hw}")
                                nc.sync.dma_start(
                                    xtall[:].rearrange("p (c t) -> p c t", t=hw),
                                    x_d[:, h0:h0 + hw].rearrange("(c p) t -> p c t", p=128))
                            xt = [xtall[:, c * hw:(c + 1) * hw] for c in range(NC_)]
                            tA = px.tile([1, hw], F32R, tag="tA")  # ssum -> mu
                            tB = px.tile([1, hw], F32, tag="tB")   # ssq -> va -> sd -> rr
                            tC = px.tile([1, hw], F32, tag="tC")   # mu2 ; then rr
                            for j0, wd in _chunks(hw):
                                p_a = ps2s.tile([1, 512], F32, tag="p_a")
                                p_b = ps2s.tile([1, 512], F32, tag="p_b")
                                for c in range(NC_):
                                    nc.tensor.matmul(p_a[:, :wd], ones_col[:], xt[c][:, j0:j0 + wd],
                                                     start=(c == 0), stop=(c == NC_ - 1),
                                                     skip_group_check=True)
                                    xsq_c = pxs.tile([128, 512], F32R, tag="xsq_c")
                                    nc.scalar.activation(xsq_c[:, :wd],
                                                         xt[c][:, j0:j0 + wd].bitcast(F32), AF.Square)
                                    nc.tensor.matmul(p_b[:, :wd], ones_col[:], xsq_c[:, :wd],
                                                     start=(c == 0), stop=(c == NC_ - 1),
                                                     skip_group_check=True)
                                nc.scalar.copy(tA[:, j0:j0 + wd], p_a[:, :wd])
                                nc.scalar.copy(tB[:, j0:j0 + wd], p_b[:, :wd])
                            nc.vector.tensor_scalar(tA[:], tA[:].bitcast(F32),
                                                    1.0 / C, None, op0=A.mult)  # mu
                            nc.vector.tensor_mul(tC[:], tA[:].bitcast(F32),
                                                 tA[:].bitcast(F32))            # mu2
                            nc.vector.scalar_tensor_tensor(tB[:], tB[:], 1.0 / C, tC[:],
                                                           op0=A.mult, op1=A.subtract)         # var
                            lneps = px.tile([1, 1], F32, tag="lneps")
                            nc.vector.memset(lneps[:], LN_EPS)
                            nc.scalar.activation(tB[:], tB[:], AF.Sqrt, bias=lneps[:, 0:1])    # sd
                            nc.vector.reciprocal(tC[:], tB[:])                                 # rr
                            rr, mu = tC, tA
                            for j0, wd in _chunks(hw):
                                r_b = pxb.tile([128, 512], F32, tag="r_b")
                                nc.gpsimd.partition_broadcast(r_b[:, :wd], rr[0:1, j0:j0 + wd],
                                                              channels=128)
                                for dh in range(2):
                                    pA = ps2.tile([128, 512], F32, tag="pA")
                                    for c in range(NC_):
                                        nc.tensor.matmul(pA[:, :wd],
                                                         wg[c][:, dh * 128:(dh + 1) * 128],
                                                         xt[c][:, j0:j0 + wd],
                                                         start=(c == 0), stop=False,
                                                         skip_group_check=True)
                                    # rank-1 mean correction: pA += outer(s, mu)
                                    sc = s_col0 + dh
                                    nc.tensor.matmul(pA[:, :wd],
                                                     suT[0:1, sc * 128:(sc + 1) * 128],
                                                     mu[0:1, j0:j0 + wd],
                                                     start=False, stop=True,
                                                     skip_group_check=True)
                                    k1 = pxs.tile([128, 512], F32, tag="k1")
                                    nc.vector.tensor_mul(k1[:, :wd], pA[:, :wd], r_b[:, :wd])
                                    nc.scalar.activation(out_T[dh][:, h0 + j0:h0 + j0 + wd],
                                                         k1[:, :wd], AF.Identity,
                                                         bias=su[:, u_col0 + dh:u_col0 + dh + 1])
                            if with_v:
                                with tc.tile_pool(name="ps3", bufs=2, space="PSUM") as ps3:
                                    for tch in range(hw // 128):
                                        t_idx = (h0 + tch * 128) // 128
                                        pV = ps3.tile([128, D], F32, tag="pV")
                                        for c in range(NC_):
                                            nc.tensor.matmul(pV[:],
                                                             xt[c][:, tch * 128:(tch + 1) * 128],
                                                             wv[c][:], start=(c == 0),
                                                             stop=(c == NC_ - 1),
                                                             skip_group_check=True)
                                        nc.scalar.activation(V[t_idx][:, 0:D], pV[:], AF.Identity)
                        h0 += hw
                        yield

                # k-side projection interleaved with the T-phase prefills.
                kgen = project_side(xkv_d, N, wkg, 2, 6, k_T, True,
                                    [512, 512, 512, 512, 256])
                next(kgen)      # kc0
                emit_T(0)
                next(kgen)      # kc1
                emit_T(1)
                next(kgen)      # kc2
                emit_T(2)
                next(kgen)      # kc3
                next(kgen)      # kc4
                nc.sync.dma_start(woall[:].rearrange("p (d c) -> p d c", c=C),
                                  wo_d[:].rearrange("(d p) c -> p d c", p=128))
                emit_T(3)
                qgen = project_side(xq_d, TQ, wqg, 0, 4, q_T, False, [384, 384, 384])
                for _ in qgen:
                    pass

                # ================= attention (A-phase) =================
                with tc.tile_pool(name="att", bufs=2) as att, \
                     tc.tile_pool(name="att2", bufs=2) as att2, \
                     tc.tile_pool(name="psL", bufs=2, space="PSUM") as psL, \
                     tc.tile_pool(name="pstp", bufs=2, space="PSUM") as pstp, \
                     tc.tile_pool(name="psO", bufs=1, space="PSUM") as psO, \
                     tc.tile_pool(name="psF", bufs=1, space="PSUM") as psF:
                    for g in range(NI):
                        gbr, s1m = tref.pop(g)
                        P = att.tile([128, N], F32R, tag="P")
                        for ci, (c0, wd) in enumerate(NCC):
                            pL = psL.tile([128, 512], F32, tag="pL")
                            # psum init: bias gbr via identity matmul, then the
                            # drop penalty BIG*s1m via a scaled-identity matmul.
                            nc.tensor.matmul(pL[:, :wd], ident_b[:],
                                             gbr[:, c0:c0 + wd],
                                             start=True, stop=False, skip_group_check=True)
                            nc.tensor.matmul(pL[:, :wd], ident_big[:],
                                             s1m[:, c0:c0 + wd],
                                             start=False, stop=False, skip_group_check=True)
                            for dh in range(2):
                                nc.tensor.matmul(pL[:, :wd],
                                                 q_T[dh][:, g * 128:(g + 1) * 128],
                                                 k_T[dh][:, c0:c0 + wd],
                                                 start=False, stop=(dh == 1),
                                                 skip_group_check=True)
                            nc.scalar.activation(P[:, c0:c0 + wd], pL[:, :wd], AF.Exp)
                        if DBG and g == 0:
                            nc.sync.dma_start(dbg_P[:], P[:].bitcast(F32))

                        # A @ V with unnormalized P; normalization folded
                        # post-AV. V carries a ones column so pO[:, D] = S.
                        pO = psO.tile([128, VW], F32, tag="pO")
                        for q4 in range(5):           # groups of 4 (last has 2)
                            njq = 2 if q4 == 4 else 4
                            ptp = pstp.tile([128, 512], F32R, tag="ptp")
                            for jj in range(njq):
                                j = q4 * 4 + jj
                                nc.tensor.transpose(ptp[:, jj * 128:(jj + 1) * 128],
                                                    P[:, j * 128:(j + 1) * 128], ident_r[:])
                            Pt = att2.tile([128, 512], F32R, tag="Pt")
                            nc.scalar.activation(Pt[:, :njq * 128],
                                                 ptp[:, :njq * 128].bitcast(F32),
                                                 AF.Identity)
                            for jj in range(njq):
                                j = q4 * 4 + jj
                                nc.tensor.matmul(pO[:], Pt[:, jj * 128:(jj + 1) * 128],
                                                 V[j][:], start=(j == 0),
                                                 stop=(j == NJ - 1), skip_group_check=True)
                        R = att2.tile([128, 1], F32, tag="R")
                        nc.vector.reciprocal(R[:], pO[:, D:D + 1])
                        if DBG and g == 0:
                            nc.sync.dma_start(dbg_S[:], R[:])
                        O_sb = att2.tile([128, D], F32R, tag="O_sb")
                        nc.scalar.activation(O_sb[:], pO[:, 0:D], AF.Copy, scale=R[:, 0:1])
                        if DBG and g == 0:
                            nc.sync.dma_start(dbg_O[:], O_sb[:].bitcast(F32))

                        OT = att2.tile([128, D], F32R, tag="OT")
                        ptp2 = pstp.tile([128, 512], F32R, tag="ptp")
                        for dh in range(2):
                            nc.tensor.transpose(ptp2[:, dh * 128:(dh + 1) * 128],
                                                O_sb[:, dh * 128:(dh + 1) * 128], ident_r[:])
                        nc.vector.tensor_scalar(OT[:], ptp2[:, 0:D].bitcast(F32), 1.0, None,
                                                op0=A.mult)
                        fo = att2.tile([128, C], F32, tag="fo")
                        for j0, wd in _chunks(C):
                            pF = psF.tile([128, 512], F32, tag="pF")
                            for dh in range(2):
                                nc.tensor.matmul(pF[:, :wd],
                                                 OT[:, dh * 128:(dh + 1) * 128],
                                                 wo[dh][:, j0:j0 + wd],
                                                 start=(dh == 0), stop=(dh == 1),
                                                 skip_group_check=True)
                            nc.scalar.copy(fo[:, j0:j0 + wd], pF[:, :wd])
                        nc.sync.dma_start(y_d[g * 128:(g + 1) * 128, :], fo[:])
                        # T-phase for g+4 emitted after A(g): its rotating
                        # gbr/s1m buffers (bufs=4) are free again.
                        if g + 4 < NI:
                            emit_T(g + 4)


    nc.finalize()
    return nc


def _host_inputs(inputs):
    qm = np.ascontiguousarray(inputs["query_map"].reshape(B, C, N))
    kv = np.ascontiguousarray(inputs["key_value_map"].reshape(B, C, N))
    pq = np.asarray(inputs["plucker_query"]).reshape(B, 6, N)
    pk = np.asarray(inputs["plucker_key"]).reshape(B, 6, N)
    wqt = np.ascontiguousarray(np.asarray(inputs["Wq"]).T)
    wkt = np.ascontiguousarray(np.asarray(inputs["Wk"]).T)
    wvt = np.ascontiguousarray(np.asarray(inputs["Wv"]).T)
    wot = np.ascontiguousarray(np.asarray(inputs["Wo"]).T)
    in_maps = []
    for core in range(8):
        b, h = core // 2, core % 2
        sl = slice(h * TQ, (h + 1) * TQ)
        m = {
            "xq": qm[b][:, sl],
            "xkv": kv[b],
            "dq": pq[b][0:3, sl],
            "mq": pq[b][3:6, sl],
            "dk": pk[b][0:3, :],
            "mk": pk[b][3:6, :],
            "pqt": pq[b][:, sl].T,
            "pkt": pk[b].T,
            "wqt": wqt, "wkt": wkt, "wvt": wvt, "wot": wot,
            "gq": np.asarray(inputs["ln_q_g"]).reshape(C, 1),
            "bqln": np.asarray(inputs["ln_q_b"]).reshape(C, 1),
            "gk": np.asarray(inputs["ln_k_g"]).reshape(C, 1),
            "bkln": np.asarray(inputs["ln_k_b"]).reshape(C, 1),
            "bq": np.asarray(inputs["bq"]).reshape(D, 1),
            "bk": np.asarray(inputs["bk"]).reshape(D, 1),
            f"nonce{BUILD_ID}": np.zeros((1, 1), np.float32),
        }
        in_maps.append({k: np.ascontiguousarray(v, dtype=np.float32)
                        for k, v in m.items()})
    return in_maps


def kernel(**inputs):
    if "nc" not in _CACHE:
        _CACHE["nc"] = build_nc()
    nc = _CACHE["nc"]
    in_maps = _host_inputs(inputs)
    res = run_bass_kernel_spmd(nc, in_maps, core_ids=list(range(8)))
    # bo' = bo + Wo@bv folded on host (layout-level affine fold).
    bo_row = (np.asarray(inputs["bo"]) +
              np.asarray(inputs["Wo"]) @ np.asarray(inputs["bv"])).astype(np.float32)
    out = np.zeros((B, C, N), np.float32)
    for core in range(8):
        b, h = core // 2, core % 2
        out[b][:, h * TQ:(h + 1) * TQ] = res.results[core]["y"].T
    out += bo_row[None, :, None]
    return out.reshape(B, C, H, W)

